# revision 1
# baseline (speedup 1.0000x reference)
"""DiT block kernel for TRN2, 8 NeuronCores, token-parallel sharding.

Sharding: the B*S = 4096 tokens are split over 8 cores: core c handles
batch b = c//2, sequence half = c%2 (512 query tokens). Each core
computes K/V over its batch's full 1024-token sequence (redundantly
with its pair core) so there are no collectives at all.

Layout strategy inside a core:
  - residual stream stays token-major [tok, D]
  - LN runs token-major (per-partition stats), then PE-transposes the
    normalized tiles to feature-major [D, tok]; the LN affine (g, b) is
    applied during the transpose copy-back where it is per-partition.
  - projections chain in feature-major; projections whose output must be
    token-major use the activation tile as lhsT instead of the weight.
  - softmax runs in scores-transposed layout [k, q]: exp without
    max-subtraction (scores are O(1) by construction), the sum over k
    comes from an extra ones-column appended to V's lhsT, and the
    1/sum normalization is broadcast via a K=1 matmul.
  - all matmuls run as float32r (full PE rate at free-dim >= 256).
"""

import numpy as np

import concourse.bass as bass
import concourse.bacc as bacc
import concourse.tile as tile
from concourse import mybir
from concourse.masks import make_identity

B, S, SC, D, H = 4, 1024, 256, 1024, 16
HD = D // H          # 64
FF = 4 * D           # 4096
T = 512              # local query tokens per core
P = 128
DT = D // P          # 8
QC = T // P          # 4
KC = S // P          # 8
CC = SC // P         # 2
FT = FF // P         # 32
EPS = 1e-5
N_CORES = 8

f32 = mybir.dt.float32
f32r = mybir.dt.float32r
AF = mybir.ActivationFunctionType
ALU = mybir.AluOpType


def r(ap):
    return ap.bitcast(f32r)


def build_nc(gelu_func=None, compile_hw=False):
    """Build the per-core Bass program (identical on all cores).

    compile_hw=True runs the bacc legalization/compile passes (required
    for hardware: TRN2 allows at most one semaphore wait per instruction
    and bacc's passes split/move the waits Tile emits). CoreSim runs on
    the uncompiled module, so simulation callers leave it False.
    """
    if gelu_func is None:
        gelu_func = AF.Gelu
    nc = bacc.Bacc("TRN2", target_bir_lowering=False, debug=False,
                   num_devices=N_CORES)

    dt_in = lambda name, shape: nc.dram_tensor(name, shape, f32, kind="ExternalInput")
    dt_inr = lambda name, shape: nc.dram_tensor(name, shape, f32r, kind="ExternalInput")

    xkv = dt_in("xkv", [S, D])            # reordered: local 512 tokens first
    cond = dt_in("cond", [SC, D])
    maskmul = dt_in("maskmul", [SC, 1])   # 1.0 where attended, 0.0 where masked
    w_qkvT = dt_inr("w_qkvT", [D, 3 * D])
    b_qkv = dt_in("b_qkv", [3 * D, 1])
    w_soutT = dt_inr("w_soutT", [D, D])
    b_sout = dt_inr("b_sout", [1, D])
    wqT = dt_inr("wqT", [D, D])
    bq = dt_in("bq", [D, 1])
    wkT = dt_inr("wkT", [D, D])
    bk = dt_in("bk", [D, 1])
    wvT = dt_inr("wvT", [D, D])
    bv = dt_in("bv", [D, 1])
    woT = dt_inr("woT", [D, D])
    bo = dt_inr("bo", [1, D])
    gate = dt_in("gate", [1, 1])
    w1T = dt_inr("w1T", [D, FF])
    b1 = dt_in("b1", [FF, 1])
    w2T = dt_inr("w2T", [FF, D])
    b2 = dt_inr("b2", [1, D])
    ln1_g = dt_in("ln1_g", [1, D])
    ln1_b = dt_in("ln1_b", [1, D])
    ln2x_g = dt_in("ln2x_g", [1, D])
    ln2x_b = dt_in("ln2x_b", [1, D])
    ln2c_g = dt_in("ln2c_g", [1, D])
    ln2c_b = dt_in("ln2c_b", [1, D])
    ln3_g = dt_in("ln3_g", [1, D])
    ln3_b = dt_in("ln3_b", [1, D])

    out = nc.dram_tensor("out", [T, D], f32, kind="ExternalOutput")

    from contextlib import ExitStack
    with tile.TileContext(nc) as tc, ExitStack() as ctx:
        ec = ctx.enter_context
        con = ec(tc.tile_pool(name="con", bufs=1))
        fm = ec(tc.tile_pool(name="fm", bufs=8))        # xs_f -> h (low half)
        kf = ec(tc.tile_pool(name="kf", bufs=8))        # k_f -> cv -> h (high half)
        a512 = ec(tc.tile_pool(name="a512", bufs=8))    # q -> sa -> xs2 -> cq -> cross -> xs3
        vp = ec(tc.tile_pool(name="vp", bufs=8))        # v tiles [P,16,65]
        cnp = ec(tc.tile_pool(name="cnp", bufs=8))      # cn_f
        ckp = ec(tc.tile_pool(name="ckp", bufs=8))      # ck_f
        resid = ec(tc.tile_pool(name="resid", bufs=4))  # x1 tiles; x2 updates in place
        xstr = ec(tc.tile_pool(name="xstr", bufs=1))    # residual input stream
        wrhs = ec(tc.tile_pool(name="wrhs", bufs=8))    # weight rhs stream [P,512]
        wsm = ec(tc.tile_pool(name="wsm", bufs=2))      # weight column-block stream [P,8,128]
        expp = ec(tc.tile_pool(name="expp", bufs=2))    # exp(scores_T) tiles
        lnin = ec(tc.tile_pool(name="lnin", bufs=2))    # LN input stream
        lnn = ec(tc.tile_pool(name="lnn", bufs=1))      # LN normalize scratch
        osb = ec(tc.tile_pool(name="osb", bufs=1))      # output staging
        smal = ec(tc.tile_pool(name="smal", bufs=4))    # stats etc.
        rcp = ec(tc.tile_pool(name="rcp", bufs=1))      # softmax 1/sum rows
        pmm = ec(tc.tile_pool(name="pmm", bufs=6, space="PSUM"))
        ptr = ec(tc.tile_pool(name="ptr", bufs=2, space="PSUM"))
        if True:
            # ---- constants ----
            ident = con.tile([P, P], f32, tag="ident")
            make_identity(nc, ident)
            ones_f32 = con.tile([P, 1], f32, tag="ones_f32")
            nc.vector.memset(ones_f32, 1.0)
            # memset can't write f32r; produce rounded ones via ACT copies
            ones_col = con.tile([P, 1], f32r, tag="ones_col")
            nc.scalar.activation(ones_col, ones_f32, AF.Copy)
            ones_row = con.tile([1, P], f32r, tag="ones_row")
            nc.scalar.activation(ones_row, ones_f32[0:1, 0:1].to_broadcast([1, P]),
                                 AF.Copy)
            eps_t = con.tile([P, 1], f32, tag="eps")
            nc.vector.memset(eps_t, EPS)

            def col_view(ap_2d, m):
                # (m*P, 1) dram tensor -> [P, m] sbuf tile
                return ap_2d[:, 0:1].rearrange("(m p) 1 -> p m", p=P)

            bqkv_sb = con.tile([P, 24], f32, tag="bqkv")
            nc.sync.dma_start(out=bqkv_sb, in_=col_view(b_qkv, 24))
            b1_sb = con.tile([P, 32], f32, tag="b1")
            nc.sync.dma_start(out=b1_sb, in_=col_view(b1, 32))
            bq_sb = con.tile([P, 8], f32, tag="bq")
            nc.sync.dma_start(out=bq_sb, in_=col_view(bq, 8))
            bk_sb = con.tile([P, 8], f32, tag="bk")
            nc.sync.dma_start(out=bk_sb, in_=col_view(bk, 8))
            bv_sb = con.tile([P, 8], f32, tag="bv")
            nc.sync.dma_start(out=bv_sb, in_=col_view(bv, 8))
            mask_sb = con.tile([P, CC], f32, tag="mask")
            nc.sync.dma_start(out=mask_sb, in_=col_view(maskmul, CC))

            lng = {}
            for nm, g_t, b_t in (("ln1", ln1_g, ln1_b), ("ln2x", ln2x_g, ln2x_b),
                                 ("ln2c", ln2c_g, ln2c_b), ("ln3", ln3_g, ln3_b)):
                gt = con.tile([P, DT], f32, tag=f"{nm}_g")
                bt = con.tile([P, DT], f32, tag=f"{nm}_b")
                nc.sync.dma_start(out=gt, in_=g_t[0:1, :].rearrange("1 (m p) -> p m", p=P))
                nc.sync.dma_start(out=bt, in_=b_t[0:1, :].rearrange("1 (m p) -> p m", p=P))
                lng[nm] = (gt, bt)

            # row-vector biases for the rank-1 bias matmuls
            bsout_sb = con.tile([1, D], f32r, tag="bsout")
            nc.sync.dma_start(out=bsout_sb, in_=b_sout[0:1, :])
            bo_sb = con.tile([1, D], f32r, tag="bo")
            nc.sync.dma_start(out=bo_sb, in_=bo[0:1, :])
            b2_sb = con.tile([1, D], f32r, tag="b2")
            nc.sync.dma_start(out=b2_sb, in_=b2[0:1, :])

            # tanh(gate), broadcast to a per-partition scalar column
            tg = con.tile([P, 1], f32, tag="tg")
            nc.gpsimd.dma_start(out=tg, in_=gate[:, :].to_broadcast([P, 1]))
            t_col = con.tile([P, 1], f32, tag="t_col")
            nc.scalar.activation(t_col, tg, AF.Tanh)

            # Advance every engine's clock past the constant setup so later
            # instructions never need a dedicated semaphore wait on a const
            # producer (the ISA allows very few waits per instruction).
            tc.strict_bb_all_engine_barrier()

            # ---- helpers ----
            def ln_tile(x_ap, gt, bt, j_targets, inplace=False):
                """LayerNorm one token-major [P, D] tile (x_ap in SBUF),
                then transpose into feature-major targets:
                j_targets[j] = destination AP [P, 128] for d-tile j.
                inplace=True overwrites x_ap with the normalized values
                (only safe for streamed inputs that are dead afterwards)."""
                stats = smal.tile([P, 2, 6], f32, tag="stats")
                nc.vector.bn_stats(out=stats[:, 0, :], in_=x_ap[:, 0:512])
                nc.vector.bn_stats(out=stats[:, 1, :], in_=x_ap[:, 512:1024])
                mv = smal.tile([P, 2], f32, tag="mv")
                nc.vector.bn_aggr(out=mv, in_=stats)
                sd = smal.tile([P, 1], f32, tag="sd")
                nc.scalar.activation(sd, mv[:, 1:2], AF.Sqrt, bias=eps_t)
                nc.vector.reciprocal(sd, sd)
                # Always normalize into a scratch tile: an in-place update
                # makes the downstream transpose wait on both the DMA and the
                # DVE producers, exceeding the LDWEIGHTS sync-wait limit.
                xn = lnn.tile([P, D], f32, tag="ln_n")
                nc.vector.tensor_scalar(xn, x_ap, mv[:, 0:1], sd,
                                        ALU.subtract, ALU.mult)
                for j in range(DT):
                    ps_t = ptr.tile([P, P], f32, tag="ptr")
                    nc.tensor.transpose(ps_t, xn[:, j * P:(j + 1) * P], ident)
                    nc.vector.tensor_scalar(j_targets[j], ps_t,
                                            gt[:, j:j + 1], bt[:, j:j + 1],
                                            ALU.mult, ALU.add)

            # ================= Phase A: LN1(xkv) -> xs_f =================
            g1, b1t = lng["ln1"]
            xs_f = [fm.tile([P, S], f32r, tag="fm", name=f"xs_f_{j}") for j in range(DT)]
            for i in range(KC):
                xt = lnin.tile([P, D], f32, tag="ln_in")
                nc.sync.dma_start(out=xt, in_=xkv[i * P:(i + 1) * P, :])
                ln_tile(xt, g1, b1t,
                        [xs_f[j][:, i * P:(i + 1) * P] for j in range(DT)],
                        inplace=True)

            # ================= Phase B: QKV projections =================
            # K feature-major [dout, 1024]
            k_f = [kf.tile([P, S], f32r, tag="kf", name=f"k_f_{j}") for j in range(DT)]
            for j in range(DT):
                ps0 = pmm.tile([P, T], f32, tag="pmm")
                ps1 = pmm.tile([P, T], f32, tag="pmm")
                wc = wsm.tile([P, DT, P], f32r, tag="wsm", name="wc_k")
                _wsrc = w_qkvT[:, (DT + j) * P:(DT + j + 1) * P].rearrange("(i p) c -> p i c", p=P)
                nc.sync.dma_start(out=wc[:, 0:4, :], in_=_wsrc[:, 0:4, :])
                nc.sync.dma_start(out=wc[:, 4:8, :], in_=_wsrc[:, 4:8, :])
                for i in range(DT):
                    nc.tensor.matmul(ps0, r(wc[:, i, :]), r(xs_f[i][:, 0:T]),
                                     start=(i == 0), stop=(i == DT - 1))
                    nc.tensor.matmul(ps1, r(wc[:, i, :]), r(xs_f[i][:, T:S]),
                                     start=(i == 0), stop=(i == DT - 1))
                bias = bqkv_sb[:, DT + j:DT + j + 1]
                nc.scalar.activation(k_f[j][:, 0:T], ps0, AF.Identity, bias=bias)
                nc.scalar.activation(k_f[j][:, T:S], ps1, AF.Identity, bias=bias)

            # Q feature-major [dout, 512] (local tokens are xkv rows 0..511)
            q_f = [None] * DT
            for j in range(DT):
                ps0 = pmm.tile([P, T], f32, tag="pmm")
                wc = wsm.tile([P, DT, P], f32r, tag="wsm", name="wc_q")
                _wsrc = w_qkvT[:, j * P:(j + 1) * P].rearrange("(i p) c -> p i c", p=P)
                nc.sync.dma_start(out=wc[:, 0:4, :], in_=_wsrc[:, 0:4, :])
                nc.sync.dma_start(out=wc[:, 4:8, :], in_=_wsrc[:, 4:8, :])
                for i in range(DT):
                    nc.tensor.matmul(ps0, r(wc[:, i, :]), r(xs_f[i][:, 0:T]),
                                     start=(i == 0), stop=(i == DT - 1))
                q_f[j] = a512.tile([P, T], f32r, tag="a512", name=f"q_f_{j}")
                nc.scalar.activation(q_f[j], ps0, AF.Identity,
                                     bias=bqkv_sb[:, j:j + 1])

            # V token-major, packed [ktok, 16 heads, 64+1] with a ones column
            # (no bias here: the v bias is added after softmax-normalization)
            v_t = [vp.tile([P, H, HD + 1], f32r, tag="vp", name=f"v_t_{c}") for c in range(KC)]
            for c in range(KC):
                # ones column via ACT so v_t has a single producing engine
                nc.scalar.activation(v_t[c][:, :, HD:HD + 1],
                                     ones_col.to_broadcast([P, H, 1]), AF.Copy)
            for half in range(2):
                wv_rhs = []
                for i in range(DT):
                    wt = wrhs.tile([P, T], f32r, tag="wrhs")
                    nc.sync.dma_start(
                        out=wt, in_=w_qkvT[i * P:(i + 1) * P,
                                           2 * D + half * T:2 * D + (half + 1) * T])
                    wv_rhs.append(wt)
                for c in range(KC):
                    ps0 = pmm.tile([P, T], f32, tag="pmm")
                    for i in range(DT):
                        nc.tensor.matmul(ps0, r(xs_f[i][:, c * P:(c + 1) * P]),
                                         r(wv_rhs[i]),
                                         start=(i == 0), stop=(i == DT - 1))
                    ps_v = ps0.rearrange("p (h d) -> p h d", h=8)
                    nc.scalar.activation(
                        v_t[c][:, half * 8:(half + 1) * 8, 0:HD], ps_v, AF.Copy)

            # cond-side work (independent of x): LN2c -> cn_f, then ck
            g2c, b2c = lng["ln2c"]
            cn_f = [cnp.tile([P, SC], f32r, tag="cnp", name=f"cn_f_{j}") for j in range(DT)]
            for i in range(CC):
                ct = lnin.tile([P, D], f32, tag="ln_in")
                nc.sync.dma_start(out=ct, in_=cond[i * P:(i + 1) * P, :])
                ln_tile(ct, g2c, b2c,
                        [cn_f[j][:, i * P:(i + 1) * P] for j in range(DT)],
                        inplace=True)

            ck_f = [None] * DT
            for j in range(DT):
                ps0 = pmm.tile([P, SC], f32, tag="pmm")
                wc = wsm.tile([P, DT, P], f32r, tag="wsm", name="wc_ck")
                _wsrc = wkT[:, j * P:(j + 1) * P].rearrange("(i p) c -> p i c", p=P)
                nc.sync.dma_start(out=wc[:, 0:4, :], in_=_wsrc[:, 0:4, :])
                nc.sync.dma_start(out=wc[:, 4:8, :], in_=_wsrc[:, 4:8, :])
                for i in range(DT):
                    nc.tensor.matmul(ps0, r(wc[:, i, :]), r(cn_f[i]),
                                     start=(i == 0), stop=(i == DT - 1))
                ck_f[j] = ckp.tile([P, SC], f32r, tag="ckp", name=f"ck_f_{j}")
                nc.scalar.activation(ck_f[j], ps0, AF.Identity,
                                     bias=bk_sb[:, j:j + 1])

            # ================= Phase C: self-attention =================
            sa_f = [None] * DT
            for h in range(H):
                dtile, poff = h // 2, (h % 2) * HD
                ps_av = pmm.tile([P, T], f32, tag="pmm")
                for c in range(KC):
                    ps_s = pmm.tile([P, T], f32, tag="pmm")
                    nc.tensor.matmul(
                        ps_s,
                        r(k_f[dtile][poff:poff + HD, c * P:(c + 1) * P]),
                        r(q_f[dtile][poff:poff + HD, :]),
                        start=True, stop=True)
                    ex = expp.tile([P, T], f32r, tag="expp")
                    nc.scalar.activation(ex, ps_s, AF.Exp, scale=0.125)
                    nc.tensor.matmul(ps_av[0:HD + 1, :], r(v_t[c][:, h, :]), r(ex),
                                     start=(c == 0), stop=(c == KC - 1))
                recip = rcp.tile([1, T], f32r, tag="recip")
                with nc.allow_low_precision(reason="softmax 1/sum in f32r"):
                    nc.vector.reciprocal(recip, ps_av[HD:HD + 1, :])
                ps_rb = pmm.tile([P, T], f32, tag="pmm")
                nc.tensor.matmul(ps_rb[0:HD, :], r(ones_row[0:1, 0:HD]), r(recip),
                                 start=True, stop=True)
                rb = rcp.tile([HD, T], f32, tag="rb")
                nc.scalar.activation(rb, ps_rb[0:HD, :], AF.Copy)
                if poff == 0:
                    sa_f[dtile] = a512.tile([P, T], f32r, tag="a512", name=f"sa_f_{dtile}")
                dst = sa_f[dtile][poff:poff + HD, :]
                nc.vector.tensor_tensor(dst, ps_av[0:HD, :], rb, ALU.mult)
                nc.vector.tensor_scalar_add(
                    dst, dst, bqkv_sb[poff:poff + HD, 2 * DT + dtile:2 * DT + dtile + 1])

            # self-attn out-proj + residual -> x1 (token-major)
            x1 = [resid.tile([P, D], f32, tag="resid", name=f"x1_{qc}") for qc in range(QC)]
            for dh in range(2):
                w_rhs = []
                for i in range(DT):
                    wt = wrhs.tile([P, T], f32r, tag="wrhs")
                    nc.sync.dma_start(
                        out=wt, in_=w_soutT[i * P:(i + 1) * P, dh * T:(dh + 1) * T])
                    w_rhs.append(wt)
                pss = [pmm.tile([P, T], f32, tag="pmm", name=f"pss_{qc}") for qc in range(QC)]
                for i in range(DT):
                    for qc in range(QC):
                        nc.tensor.matmul(pss[qc],
                                         r(sa_f[i][:, qc * P:(qc + 1) * P]),
                                         r(w_rhs[i]),
                                         start=(i == 0), stop=False)
                for qc in range(QC):
                    nc.tensor.matmul(pss[qc], r(ones_row[0:1, :]),
                                     r(bsout_sb[0:1, dh * T:(dh + 1) * T]),
                                     start=False, stop=True)
                    xin = xstr.tile([P, T], f32, tag="xstr")
                    nc.sync.dma_start(
                        out=xin, in_=xkv[qc * P:(qc + 1) * P, dh * T:(dh + 1) * T])
                    nc.vector.tensor_tensor(x1[qc][:, dh * T:(dh + 1) * T],
                                            pss[qc], xin, ALU.add)

            # ================= Phase D: cross-attention =================
            g2x, b2x = lng["ln2x"]
            xs2_f = [vp.tile([P, T], f32r, tag="vp", name=f"xs2_f_{j}") for j in range(DT)]
            for qc in range(QC):
                ln_tile(x1[qc], g2x, b2x,
                        [xs2_f[j][:, qc * P:(qc + 1) * P] for j in range(DT)])

            # cq (feature-major), ck (feature-major over cond tokens)
            cq_f = [None] * DT
            for j in range(DT):
                ps0 = pmm.tile([P, T], f32, tag="pmm")
                wc = wsm.tile([P, DT, P], f32r, tag="wsm", name="wc_cq")
                _wsrc = wqT[:, j * P:(j + 1) * P].rearrange("(i p) c -> p i c", p=P)
                nc.sync.dma_start(out=wc[:, 0:4, :], in_=_wsrc[:, 0:4, :])
                nc.sync.dma_start(out=wc[:, 4:8, :], in_=_wsrc[:, 4:8, :])
                for i in range(DT):
                    nc.tensor.matmul(ps0, r(wc[:, i, :]), r(xs2_f[i]),
                                     start=(i == 0), stop=(i == DT - 1))
                cq_f[j] = a512.tile([P, T], f32r, tag="a512", name=f"cq_f_{j}")
                nc.scalar.activation(cq_f[j], ps0, AF.Identity,
                                     bias=bq_sb[:, j:j + 1])
            # cv token-major [cond_tok, D] (bias folded in after normalization)
            cv_t = [kf.tile([P, D], f32r, tag="kf", name=f"cv_t_{c}") for c in range(CC)]
            for half in range(2):
                wv_rhs = []
                for i in range(DT):
                    wt = wrhs.tile([P, T], f32r, tag="wrhs")
                    nc.sync.dma_start(
                        out=wt, in_=wvT[i * P:(i + 1) * P, half * T:(half + 1) * T])
                    wv_rhs.append(wt)
                for c in range(CC):
                    ps0 = pmm.tile([P, T], f32, tag="pmm")
                    for i in range(DT):
                        nc.tensor.matmul(ps0, r(cn_f[i][:, c * P:(c + 1) * P]),
                                         r(wv_rhs[i]),
                                         start=(i == 0), stop=(i == DT - 1))
                    nc.scalar.activation(cv_t[c][:, half * T:(half + 1) * T],
                                         ps0, AF.Copy)

            # cross scores (transposed), masked exp, sum
            exc = [None] * CC
            ps_sum = pmm.tile([P, T], f32, tag="pmm")
            for c in range(CC):
                ps_a = pmm.tile([P, T], f32, tag="pmm")
                for i in range(DT):
                    nc.tensor.matmul(ps_a, r(ck_f[i][:, c * P:(c + 1) * P]),
                                     r(cq_f[i]),
                                     start=(i == 0), stop=(i == DT - 1))
                exc[c] = expp.tile([P, T], f32r, tag="expp", name=f"exc_{c}")
                nc.scalar.activation(exc[c], ps_a, AF.Exp, scale=1.0 / 32.0)
                nc.vector.tensor_scalar_mul(exc[c], exc[c], mask_sb[:, c:c + 1])
                nc.tensor.matmul(ps_sum[0:1, :], r(ones_col), r(exc[c]),
                                 start=(c == 0), stop=(c == CC - 1))
            recip = rcp.tile([1, T], f32r, tag="recip")
            with nc.allow_low_precision(reason="softmax 1/sum in f32r"):
                nc.vector.reciprocal(recip, ps_sum[0:1, :])
            ps_rb = pmm.tile([P, T], f32, tag="pmm")
            nc.tensor.matmul(ps_rb, r(ones_row), r(recip), start=True, stop=True)
            rb_c = rcp.tile([P, T], f32, tag="rb")
            nc.scalar.activation(rb_c, ps_rb, AF.Copy)

            # cross AV -> cross_f (feature-major), normalize + v-bias
            cross_f = [None] * DT
            for j in range(DT):
                ps_c = pmm.tile([P, T], f32, tag="pmm")
                for c in range(CC):
                    nc.tensor.matmul(ps_c, r(cv_t[c][:, j * P:(j + 1) * P]),
                                     r(exc[c]),
                                     start=(c == 0), stop=(c == CC - 1))
                cross_f[j] = a512.tile([P, T], f32r, tag="a512", name=f"cross_f_{j}")
                nc.vector.tensor_tensor(cross_f[j], ps_c, rb_c, ALU.mult)
                nc.vector.tensor_scalar_add(cross_f[j], cross_f[j],
                                            bv_sb[:, j:j + 1])

            # wo proj, gate, residual -> x2 (in place over x1)
            x2 = x1
            for dh in range(2):
                w_rhs = []
                for i in range(DT):
                    wt = wrhs.tile([P, T], f32r, tag="wrhs")
                    nc.sync.dma_start(
                        out=wt, in_=woT[i * P:(i + 1) * P, dh * T:(dh + 1) * T])
                    w_rhs.append(wt)
                pss = [pmm.tile([P, T], f32, tag="pmm", name=f"pss_{qc}") for qc in range(QC)]
                for i in range(DT):
                    for qc in range(QC):
                        nc.tensor.matmul(pss[qc],
                                         r(cross_f[i][:, qc * P:(qc + 1) * P]),
                                         r(w_rhs[i]),
                                         start=(i == 0), stop=False)
                for qc in range(QC):
                    nc.tensor.matmul(pss[qc], r(ones_row[0:1, :]),
                                     r(bo_sb[0:1, dh * T:(dh + 1) * T]),
                                     start=False, stop=True)
                    sl = (slice(None), slice(dh * T, (dh + 1) * T))
                    nc.vector.scalar_tensor_tensor(
                        x2[qc][sl], pss[qc], t_col, x1[qc][sl],
                        ALU.mult, ALU.add)

            # ================= Phase E: FFN =================
            g3, b3 = lng["ln3"]
            xs3_f = [a512.tile([P, T], f32r, tag="a512", name=f"xs3_f_{j}") for j in range(DT)]
            for qc in range(QC):
                ln_tile(x2[qc], g3, b3,
                        [xs3_f[j][:, qc * P:(qc + 1) * P] for j in range(DT)])

            # up-proj + gelu -> h tiles, packed two ff-tiles per slot
            h_lo = [fm.tile([P, 2, T], f32r, tag="fm", name=f"h_lo_{j}") for j in range(8)]
            h_hi = [kf.tile([P, 2, T], f32r, tag="kf", name=f"h_hi_{j}") for j in range(8)]

            def h_slice(f, qc=None):
                t_ = h_lo[f // 2] if f < 16 else h_hi[(f - 16) // 2]
                if qc is None:
                    return t_[:, f % 2, :]
                return t_[:, f % 2, qc * P:(qc + 1) * P]

            for f in range(FT):
                ps0 = pmm.tile([P, T], f32, tag="pmm")
                wc = wsm.tile([P, DT, P], f32r, tag="wsm", name="wc_up")
                _wsrc = w1T[:, f * P:(f + 1) * P].rearrange("(i p) c -> p i c", p=P)
                nc.sync.dma_start(out=wc[:, 0:4, :], in_=_wsrc[:, 0:4, :])
                nc.sync.dma_start(out=wc[:, 4:8, :], in_=_wsrc[:, 4:8, :])
                for i in range(DT):
                    nc.tensor.matmul(ps0, r(wc[:, i, :]), r(xs3_f[i]),
                                     start=(i == 0), stop=(i == DT - 1))
                nc.scalar.activation(h_slice(f), ps0, gelu_func,
                                     bias=b1_sb[:, f:f + 1])

            # down-proj + residual -> out
            for dh in range(2):
                pss = [pmm.tile([P, T], f32, tag="pmm", name=f"pss_{qc}") for qc in range(QC)]
                for f in range(FT):
                    wt = wrhs.tile([P, T], f32r, tag="wrhs")
                    nc.sync.dma_start(
                        out=wt[:, 0:256],
                        in_=w2T[f * P:(f + 1) * P, dh * T:dh * T + 256])
                    nc.sync.dma_start(
                        out=wt[:, 256:T],
                        in_=w2T[f * P:(f + 1) * P, dh * T + 256:(dh + 1) * T])
                    for qc in range(QC):
                        nc.tensor.matmul(pss[qc], r(h_slice(f, qc)), r(wt),
                                         start=(f == 0), stop=False)
                for qc in range(QC):
                    nc.tensor.matmul(pss[qc], r(ones_row[0:1, :]),
                                     r(b2_sb[0:1, dh * T:(dh + 1) * T]),
                                     start=False, stop=True)
                    ot = osb.tile([P, T], f32, tag="osb")
                    nc.vector.tensor_tensor(
                        ot, pss[qc], x2[qc][:, dh * T:(dh + 1) * T], ALU.add)
                    nc.sync.dma_start(
                        out=out[qc * P:(qc + 1) * P, dh * T:(dh + 1) * T], in_=ot)

    if compile_hw:
        nc.compile()
    return nc


def make_in_maps(inputs):
    """Host-side sharding/layout prep. inputs: full arrays as in reference."""
    f = np.float32
    x = np.asarray(inputs["x"], f)
    cond = np.asarray(inputs["cond"], f)
    cmask = np.asarray(inputs["cond_mask"])
    g = lambda k: np.ascontiguousarray(np.asarray(inputs[k], f))
    tr = lambda k: np.ascontiguousarray(np.asarray(inputs[k], f).T)

    shared = {
        "w_qkvT": tr("sa_in_w"),
        "b_qkv": g("sa_in_b").reshape(3 * D, 1),
        "w_soutT": tr("sa_out_w"),
        "b_sout": g("sa_out_b").reshape(1, D),
        "wqT": tr("wq"), "bq": g("bq").reshape(D, 1),
        "wkT": tr("wk"), "bk": g("bk").reshape(D, 1),
        "wvT": tr("wv"), "bv": g("bv").reshape(D, 1),
        "woT": tr("wo"), "bo": g("bo").reshape(1, D),
        "gate": g("gate").reshape(1, 1),
        "w1T": tr("w1"), "b1": g("b1").reshape(FF, 1),
        "w2T": tr("w2"), "b2": g("b2").reshape(1, D),
        "ln1_g": g("ln1_g").reshape(1, D), "ln1_b": g("ln1_b").reshape(1, D),
        "ln2x_g": g("ln2x_g").reshape(1, D), "ln2x_b": g("ln2x_b").reshape(1, D),
        "ln2c_g": g("ln2c_g").reshape(1, D), "ln2c_b": g("ln2c_b").reshape(1, D),
        "ln3_g": g("ln3_g").reshape(1, D), "ln3_b": g("ln3_b").reshape(1, D),
    }
    in_maps = []
    for c in range(N_CORES):
        b, half = c // 2, c % 2
        loc = x[b, half * T:(half + 1) * T]
        oth = x[b, (1 - half) * T:(2 - half) * T]
        m = dict(shared)
        m["xkv"] = np.ascontiguousarray(np.concatenate([loc, oth], axis=0))
        m["cond"] = np.ascontiguousarray(cond[b])
        m["maskmul"] = (cmask[b] != 0).astype(f).reshape(SC, 1)
        in_maps.append(m)
    return in_maps


_CACHED_NC = None


def kernel(**inputs):
    from concourse.bass_utils import run_bass_kernel_spmd
    global _CACHED_NC
    if _CACHED_NC is None:
        _CACHED_NC = build_nc(compile_hw=True)
    in_maps = make_in_maps(inputs)
    res = run_bass_kernel_spmd(_CACHED_NC, in_maps, list(range(N_CORES)))
    out = np.empty((B, S, D), np.float32)
    for c in range(N_CORES):
        b, half = c // 2, c % 2
        out[b, half * T:(half + 1) * T] = res.results[c]["out"]
    return out



# revision 4
# speedup vs baseline: 1.6627x; 1.6627x over previous
"""DiT block kernel for TRN2, 8 NeuronCores, token-parallel sharding.

Sharding: the B*S = 4096 tokens are split over 8 cores: core c handles
batch b = c//2, sequence half = c%2 (512 query tokens). Each core
computes K/V over its batch's full 1024-token sequence (redundantly
with its pair core) so there are no collectives at all.

v2: fp8 (e4m3) DoubleRow matmuls for every large GEMM. DoubleRow
processes two [128,M]x[128,N] products per instruction at 0.5
cycles/row (4x the f32r rate); operand "pair tiles" [128, 2, N] hold
the two contraction planes along the free axis. Weights are quantized
on the host at scale 512 (descale 2^-9 folds into the PSUM-drain
stage); the FFN weights additionally carry an fp8 residual plane
(W ~= W_hi + W_lo at the same scale, accumulated in the same PSUM
group) which cuts effective weight-quantization error to ~0.2%.
Activations quantize to fp8 at scale 1 (e4m3 covers their dynamic
range); attention q/k stay f32r for the scores; softmax uses the
no-max exp trick with the sum taken via an fp8 ones-column in V, so
the fp8 quantization of exp() largely cancels in the normalization.
The v-bias of both attentions folds into the following projection
bias on the host (softmax weights sum to 1, so it is exact).

Layout strategy inside a core:
  - residual stream stays token-major [tok, D] in f32
  - LN: bn_stats in f32 (DVE), normalize to bf16 (GPSIMD), PE
    transpose in bf16 (1 cycle/row), per-partition LN affine + fp8
    quantize fused into the transpose drain (ACT/DVE alternating).
  - PSUM can only be drained by ACT/DVE; GPSIMD takes SBUF->SBUF work.
"""

import numpy as np
import ml_dtypes

import concourse.bass as bass
import concourse.bacc as bacc
import concourse.tile as tile
from concourse import mybir
from concourse.masks import make_identity

B, S, SC, D, H = 4, 1024, 256, 1024, 16
HD = D // H          # 64
FF = 4 * D           # 4096
T = 512              # local query tokens per core
P = 128
DT = D // P          # 8
DP = DT // 2         # 4 d-tile pairs
QC = T // P          # 4
KC = S // P          # 8
CC = SC // P         # 2
FT = FF // P         # 32
FP = FT // 2         # 16 ff-tile pairs
EPS = 1e-5
N_CORES = 8

SW = 512.0           # host-side weight quantization scale
DSC = 1.0 / SW       # descale folded into PSUM-drain stages
SSA = 16.0           # fp8 scale on softmax-normalized outputs (sa, cross)

LHS_SZ = 8192        # _pack_lhsT bytes/partition for a 1024x1024 weight
RHS_SZ = 8192        # _pack_rhs bytes/partition for a 1024x1024 weight
W1_SZ = 32768        # per hi/lo plane
W2_SZ = 32768

f32 = mybir.dt.float32
f32r = mybir.dt.float32r
bf16 = mybir.dt.bfloat16
fp8 = mybir.dt.float8e4
AF = mybir.ActivationFunctionType
ALU = mybir.AluOpType
DR = mybir.MatmulPerfMode.DoubleRow


def r(ap):
    return ap.bitcast(f32r)


def build_nc(gelu_func=None, compile_hw=False):
    """Build the per-core Bass program (identical on all cores)."""
    if gelu_func is None:
        gelu_func = AF.Gelu
    nc = bacc.Bacc("TRN2", target_bir_lowering=False, debug=False,
                   num_devices=N_CORES)

    dt_in = lambda name, shape: nc.dram_tensor(name, shape, f32, kind="ExternalInput")
    dt_in8 = lambda name, shape: nc.dram_tensor(name, shape, fp8, kind="ExternalInput")
    dt_inr = lambda name, shape: nc.dram_tensor(name, shape, f32r, kind="ExternalInput")

    xkv = dt_in("xkv", [S, D])            # reordered: local 512 tokens first
    cond = dt_in("cond", [SC, D])
    maskmul = dt_in("maskmul", [SC, 1])   # 1.0 where attended, 0.0 masked
    # fp8 weight packs, already in per-tile SBUF layout (see make_in_maps)
    wqkv_pk = dt_in8("wqkv_pk", [P, 3 * LHS_SZ])   # K | Q | V
    wsout_pk = dt_in8("wsout_pk", [P, RHS_SZ])
    wq_pk = dt_in8("wq_pk", [P, LHS_SZ])
    wk_pk = dt_in8("wk_pk", [P, LHS_SZ])
    wv_pk = dt_in8("wv_pk", [P, RHS_SZ])
    wo_pk = dt_in8("wo_pk", [P, RHS_SZ])
    w1_pk = dt_in8("w1_pk", [P, 2 * W1_SZ])        # hi | lo
    w2_pk = dt_in8("w2_pk", [P, 2 * W2_SZ])        # hi | lo
    b_qkv = dt_in("b_qkv", [3 * D, 1])
    b_sout = dt_inr("b_sout", [1, D])     # pre-scaled by SW*SSA, v-bias folded
    bq = dt_in("bq", [D, 1])
    bk = dt_in("bk", [D, 1])
    bo = dt_inr("bo", [1, D])             # pre-scaled by SW*SSA, cv-bias folded
    gate = dt_in("gate", [1, 1])
    b1 = dt_in("b1", [FF, 1])
    b2 = dt_inr("b2", [1, D])             # pre-scaled by SW
    ln1_g = dt_in("ln1_g", [1, D])
    ln1_b = dt_in("ln1_b", [1, D])
    ln2x_g = dt_in("ln2x_g", [1, D])
    ln2x_b = dt_in("ln2x_b", [1, D])
    ln2c_g = dt_in("ln2c_g", [1, D])
    ln2c_b = dt_in("ln2c_b", [1, D])
    ln3_g = dt_in("ln3_g", [1, D])
    ln3_b = dt_in("ln3_b", [1, D])

    out = nc.dram_tensor("out", [T, D], f32, kind="ExternalOutput")

    from contextlib import ExitStack
    with tile.TileContext(nc) as tc, ExitStack() as ctx:
        ec = ctx.enter_context
        con = ec(tc.tile_pool(name="con", bufs=1))
        xsp = ec(tc.tile_pool(name="xsp", bufs=4))      # xs pair tiles
        kfp = ec(tc.tile_pool(name="kfp", bufs=8))      # k_f f32r
        qfp = ec(tc.tile_pool(name="qfp", bufs=8))      # q_f f32r
        vp = ec(tc.tile_pool(name="vp", bufs=4))        # v pair tiles
        cnp = ec(tc.tile_pool(name="cnp", bufs=4))      # cn pairs
        ckp = ec(tc.tile_pool(name="ckp", bufs=4))      # ck pairs
        a512 = ec(tc.tile_pool(name="a512", bufs=8))    # sa/xs2/cq/cross/xs3
        cvp = ec(tc.tile_pool(name="cvp", bufs=1))      # cv pair tile
        hp = ec(tc.tile_pool(name="hp", bufs=16))       # FFN h pair tiles
        resid = ec(tc.tile_pool(name="resid", bufs=4))  # x1/x2 tiles f32
        xstr = ec(tc.tile_pool(name="xstr", bufs=2))    # residual input stream
        wst = ec(tc.tile_pool(name="wst", bufs=3))      # 8KB weight strips
        expp = ec(tc.tile_pool(name="expp", bufs=4))    # ex pair tiles fp8
        lnin = ec(tc.tile_pool(name="lnin", bufs=2))    # LN input stream f32
        lnn = ec(tc.tile_pool(name="lnn", bufs=2))      # LN normalize bf16
        osb = ec(tc.tile_pool(name="osb", bufs=2))      # output staging
        smal = ec(tc.tile_pool(name="smal", bufs=4))    # stats etc.
        rcp = ec(tc.tile_pool(name="rcp", bufs=2))      # softmax 1/sum rows
        pmm = ec(tc.tile_pool(name="pmm", bufs=6, space="PSUM"))
        ptr = ec(tc.tile_pool(name="ptr", bufs=2, space="PSUM"))
        if True:
            # ---- constants ----
            ident = con.tile([P, P], bf16, tag="ident")
            make_identity(nc, ident)
            ones_f32 = con.tile([P, 1], f32, tag="ones_f32")
            nc.vector.memset(ones_f32, 1.0)
            ones_col8 = con.tile([P, 1], fp8, tag="ones_col8")
            nc.scalar.activation(ones_col8, ones_f32, AF.Copy)
            ones_row = con.tile([1, P], f32r, tag="ones_row")
            nc.scalar.activation(ones_row, ones_f32[0:1, 0:1].to_broadcast([1, P]),
                                 AF.Copy)
            # row of SSA for the softmax-normalize broadcast (folds the fp8
            # output scale of sa/cross into the reciprocal broadcast)
            ssa_row = con.tile([1, P], f32r, tag="ssa_row")
            nc.scalar.activation(ssa_row, ones_f32[0:1, 0:1].to_broadcast([1, P]),
                                 AF.Copy, scale=SSA)
            eps_t = con.tile([P, 1], f32, tag="eps")
            nc.vector.memset(eps_t, EPS)

            def col_view(ap_2d, m):
                # (m*P, 1) dram tensor -> [P, m] sbuf tile
                return ap_2d[:, 0:1].rearrange("(m p) 1 -> p m", p=P)

            bqkv_sb = con.tile([P, 16], f32, tag="bqkv")    # q, k bias cols
            nc.sync.dma_start(out=bqkv_sb, in_=col_view(b_qkv, 24)[:, 0:16])
            b1_sb = con.tile([P, 32], f32, tag="b1")
            nc.sync.dma_start(out=b1_sb, in_=col_view(b1, 32))
            bq_sb = con.tile([P, 8], f32, tag="bq")
            nc.sync.dma_start(out=bq_sb, in_=col_view(bq, 8))
            bk_sb = con.tile([P, 8], f32, tag="bk")
            nc.sync.dma_start(out=bk_sb, in_=col_view(bk, 8))
            mask_sb = con.tile([P, CC], f32, tag="mask")
            nc.sync.dma_start(out=mask_sb, in_=col_view(maskmul, CC))

            lng = {}
            for nm, g_t, b_t in (("ln1", ln1_g, ln1_b), ("ln2x", ln2x_g, ln2x_b),
                                 ("ln2c", ln2c_g, ln2c_b), ("ln3", ln3_g, ln3_b)):
                gt = con.tile([P, DT], f32, tag=f"{nm}_g")
                bt = con.tile([P, DT], f32, tag=f"{nm}_b")
                nc.sync.dma_start(out=gt, in_=g_t[0:1, :].rearrange("1 (m p) -> p m", p=P))
                nc.sync.dma_start(out=bt, in_=b_t[0:1, :].rearrange("1 (m p) -> p m", p=P))
                lng[nm] = (gt, bt)

            # row-vector biases for the rank-1 bias matmuls (host pre-scaled)
            bsout_sb = con.tile([1, D], f32r, tag="bsout")
            nc.sync.dma_start(out=bsout_sb, in_=b_sout[0:1, :])
            bo_sb = con.tile([1, D], f32r, tag="bo")
            nc.sync.dma_start(out=bo_sb, in_=bo[0:1, :])
            b2_sb = con.tile([1, D], f32r, tag="b2")
            nc.sync.dma_start(out=b2_sb, in_=b2[0:1, :])

            # t_col = tanh(gate) * DSC / SSA  (post-scale needs two steps)
            tg = con.tile([P, 1], f32, tag="tg")
            nc.gpsimd.dma_start(out=tg, in_=gate[:, :].to_broadcast([P, 1]))
            tg2 = con.tile([P, 1], f32, tag="tg2")
            nc.scalar.activation(tg2, tg, AF.Tanh)
            t_col = con.tile([P, 1], f32, tag="t_col")
            nc.scalar.activation(t_col, tg2, AF.Copy, scale=DSC / SSA)

            tc.strict_bb_all_engine_barrier()

            # ---- helpers ----
            aff_rr = [0]

            def ln_tile(x_ap, gt, bt, j_targets):
                """LayerNorm one token-major [P, D] f32 tile; transpose in
                bf16 and write fp8 (affine fused) into feature-major targets:
                j_targets[j] = destination AP [P, 128] (fp8) for d-tile j."""
                stats = smal.tile([P, 2, 6], f32, tag="stats")
                nc.vector.bn_stats(out=stats[:, 0, :], in_=x_ap[:, 0:512])
                nc.vector.bn_stats(out=stats[:, 1, :], in_=x_ap[:, 512:1024])
                mv = smal.tile([P, 2], f32, tag="mv")
                nc.vector.bn_aggr(out=mv, in_=stats)
                sd = smal.tile([P, 1], f32, tag="sd")
                nc.scalar.activation(sd, mv[:, 1:2], AF.Sqrt, bias=eps_t)
                nc.vector.reciprocal(sd, sd)
                xn = lnn.tile([P, D], bf16, tag="ln_n")
                nc.gpsimd.tensor_scalar(xn, x_ap, mv[:, 0:1], sd,
                                        ALU.subtract, ALU.mult)
                for j in range(DT):
                    ps_t = ptr.tile([P, P], bf16, tag="ptr")
                    nc.tensor.transpose(ps_t, xn[:, j * P:(j + 1) * P], ident)
                    if aff_rr[0] % 2 == 0:
                        nc.scalar.activation(j_targets[j], ps_t, AF.Identity,
                                             bias=bt[:, j:j + 1],
                                             scale=gt[:, j:j + 1])
                    else:
                        nc.vector.tensor_scalar(j_targets[j], ps_t,
                                                gt[:, j:j + 1], bt[:, j:j + 1],
                                                ALU.mult, ALU.add)
                    aff_rr[0] += 1

            # ============ Phase A: LN1(xkv) -> xs pair tiles (fp8) ============
            g1, b1t = lng["ln1"]
            xs_p = [xsp.tile([P, 2, S], fp8, tag="xsp", name=f"xs_p_{i}")
                    for i in range(DP)]
            for i in range(KC):
                xt = lnin.tile([P, D], f32, tag="ln_in")
                nc.sync.dma_start(out=xt, in_=xkv[i * P:(i + 1) * P, :])
                ln_tile(xt, g1, b1t,
                        [xs_p[j // 2][:, j % 2, i * P:(i + 1) * P]
                         for j in range(DT)])

            # cond-side LN (independent of x): LN2c -> cn pairs
            g2c, b2c = lng["ln2c"]
            cn_p = [cnp.tile([P, 2, SC], fp8, tag="cnp", name=f"cn_p_{i}")
                    for i in range(DP)]
            for i in range(CC):
                ct = lnin.tile([P, D], f32, tag="ln_in")
                nc.sync.dma_start(out=ct, in_=cond[i * P:(i + 1) * P, :])
                ln_tile(ct, g2c, b2c,
                        [cn_p[j // 2][:, j % 2, i * P:(i + 1) * P]
                         for j in range(DT)])

            # ============ Phase B: QKV projections (fp8 DoubleRow) ============
            wk_st = wst.tile([P, DT, DP, 2, P], fp8, tag="wst", name="wk_st")
            nc.sync.dma_start(out=wk_st, in_=wqkv_pk[:, 0:LHS_SZ].rearrange(
                "p (j i t c) -> p j i t c", j=DT, i=DP, t=2))
            wq_st = wst.tile([P, DT, DP, 2, P], fp8, tag="wst", name="wq_st")
            nc.sync.dma_start(out=wq_st, in_=wqkv_pk[:, LHS_SZ:2 * LHS_SZ].rearrange(
                "p (j i t c) -> p j i t c", j=DT, i=DP, t=2))
            wv_st = wst.tile([P, 2, DP, 2, 4 * P], fp8, tag="wst", name="wv_st")
            nc.sync.dma_start(out=wv_st, in_=wqkv_pk[:, 2 * LHS_SZ:3 * LHS_SZ].rearrange(
                "p (h i t c) -> p h i t c", h=2, i=DP, t=2))

            # K feature-major f32r [dout, 1024]
            k_f = [kfp.tile([P, S], f32r, tag="kfp", name=f"k_f_{j}")
                   for j in range(DT)]
            for j in range(DT):
                for half in range(2):
                    ps0 = pmm.tile([P, T], f32, tag="pmm")
                    for i in range(DP):
                        nc.tensor.matmul(ps0, wk_st[:, j, i, :, :],
                                         xs_p[i][:, :, half * T:(half + 1) * T],
                                         start=(i == 0), stop=(i == DP - 1),
                                         perf_mode=DR)
                    nc.scalar.activation(k_f[j][:, half * T:(half + 1) * T],
                                         ps0, AF.Identity, scale=DSC,
                                         bias=bqkv_sb[:, 8 + j:9 + j])

            # Q feature-major f32r [dout, 512] (local tokens)
            q_f = [qfp.tile([P, T], f32r, tag="qfp", name=f"q_f_{j}")
                   for j in range(DT)]
            for j in range(DT):
                ps0 = pmm.tile([P, T], f32, tag="pmm")
                for i in range(DP):
                    nc.tensor.matmul(ps0, wq_st[:, j, i, :, :],
                                     xs_p[i][:, :, 0:T],
                                     start=(i == 0), stop=(i == DP - 1),
                                     perf_mode=DR)
                nc.scalar.activation(q_f[j], ps0, AF.Identity, scale=DSC,
                                     bias=bqkv_sb[:, j:j + 1])

            # V token-major fp8 pair tiles [ktok, 2, 16 heads, 64+1 dims]
            v_p = [vp.tile([P, 2, H, HD + 1], fp8, tag="vp", name=f"v_p_{g}")
                   for g in range(KC // 2)]
            for g in range(KC // 2):
                nc.gpsimd.memset(v_p[g][:, :, :, HD:HD + 1], 1.0)
            for c in range(KC):
                for half in range(2):
                    ps0 = pmm.tile([P, T], f32, tag="pmm")
                    for i in range(DP):
                        nc.tensor.matmul(ps0, xs_p[i][:, :, c * P:(c + 1) * P],
                                         wv_st[:, half, i, :, :],
                                         start=(i == 0), stop=(i == DP - 1),
                                         perf_mode=DR)
                    ps_v = ps0.rearrange("p (h d) -> p h d", h=8)
                    nc.vector.tensor_scalar(
                        v_p[c // 2][:, c % 2, half * 8:(half + 1) * 8, 0:HD],
                        ps_v, DSC, None, ALU.mult)

            # cond-side projections: ck pairs (feature-major over cond tokens)
            wkc_st = wst.tile([P, DT, DP, 2, P], fp8, tag="wst", name="wkc_st")
            nc.sync.dma_start(out=wkc_st, in_=wk_pk[:, :].rearrange(
                "p (j i t c) -> p j i t c", j=DT, i=DP, t=2))
            ck_p = [ckp.tile([P, 2, SC], fp8, tag="ckp", name=f"ck_p_{i}")
                    for i in range(DP)]
            for j in range(DT):
                ps0 = pmm.tile([P, SC], f32, tag="pmm")
                for i in range(DP):
                    nc.tensor.matmul(ps0, wkc_st[:, j, i, :, :], cn_p[i],
                                     start=(i == 0), stop=(i == DP - 1),
                                     perf_mode=DR)
                nc.scalar.activation(ck_p[j // 2][:, j % 2, :], ps0,
                                     AF.Identity, scale=DSC,
                                     bias=bk_sb[:, j:j + 1])

            # cv pair tile (token-major over cond tokens) fp8
            wvc_st = wst.tile([P, 2, DP, 2, 4 * P], fp8, tag="wst", name="wvc_st")
            nc.sync.dma_start(out=wvc_st, in_=wv_pk[:, :].rearrange(
                "p (h i t c) -> p h i t c", h=2, i=DP, t=2))
            cv_p = cvp.tile([P, 2, D], fp8, tag="cvp")
            for c in range(CC):
                for half in range(2):
                    ps0 = pmm.tile([P, T], f32, tag="pmm")
                    for i in range(DP):
                        nc.tensor.matmul(ps0, cn_p[i][:, :, c * P:(c + 1) * P],
                                         wvc_st[:, half, i, :, :],
                                         start=(i == 0), stop=(i == DP - 1),
                                         perf_mode=DR)
                    nc.vector.tensor_scalar(
                        cv_p[:, c, half * T:(half + 1) * T], ps0,
                        DSC, None, ALU.mult)

            # ============ Phase C: self-attention ============
            # sa pair tiles (feature-major, fp8, scaled by SSA)
            sa_p = [a512.tile([P, 2, T], fp8, tag="a512", name=f"sa_p_{i}")
                    for i in range(DP)]
            for h in range(H):
                dtile, poff = h // 2, (h % 2) * HD
                ps_av = pmm.tile([P, T], f32, tag="pmm")
                for cg in range(KC // 2):
                    exg = expp.tile([P, 2, T], fp8, tag="expp")
                    for sub in range(2):
                        c = 2 * cg + sub
                        ps_s = pmm.tile([P, T], f32, tag="pmm")
                        nc.tensor.matmul(
                            ps_s,
                            r(k_f[dtile][poff:poff + HD, c * P:(c + 1) * P]),
                            r(q_f[dtile][poff:poff + HD, :]),
                            start=True, stop=True)
                        nc.scalar.activation(exg[:, sub, :], ps_s, AF.Exp,
                                             scale=0.125)
                    nc.tensor.matmul(ps_av[0:HD + 1, :],
                                     v_p[cg][:, :, h, :], exg,
                                     start=(cg == 0), stop=(cg == KC // 2 - 1),
                                     perf_mode=DR)
                recip = rcp.tile([1, T], f32r, tag="recip")
                with nc.allow_low_precision(reason="softmax 1/sum in f32r"):
                    nc.vector.reciprocal(recip, ps_av[HD:HD + 1, :])
                ps_rb = pmm.tile([P, T], f32, tag="pmm")
                nc.tensor.matmul(ps_rb[0:HD, :], r(ssa_row[0:1, 0:HD]), r(recip),
                                 start=True, stop=True)
                rb = rcp.tile([HD, T], f32, tag="rb")
                nc.vector.tensor_scalar(rb, ps_rb[0:HD, :], 1.0, None, ALU.mult)
                dst = sa_p[dtile // 2][:, dtile % 2, :]
                nc.vector.tensor_tensor(dst[poff:poff + HD, :],
                                        ps_av[0:HD, :], rb, ALU.mult)

            # self-attn out-proj + residual -> x1 (token-major f32)
            ws_st = wst.tile([P, 2, DP, 2, 4 * P], fp8, tag="wst", name="ws_st")
            nc.sync.dma_start(out=ws_st, in_=wsout_pk[:, :].rearrange(
                "p (h i t c) -> p h i t c", h=2, i=DP, t=2))
            x1 = [resid.tile([P, D], f32, tag="resid", name=f"x1_{qc}")
                  for qc in range(QC)]
            for dh in range(2):
                pss = [pmm.tile([P, T], f32, tag="pmm", name=f"pss_{qc}")
                       for qc in range(QC)]
                for i in range(DP):
                    for qc in range(QC):
                        nc.tensor.matmul(pss[qc],
                                         sa_p[i][:, :, qc * P:(qc + 1) * P],
                                         ws_st[:, dh, i, :, :],
                                         start=(i == 0), stop=False,
                                         perf_mode=DR)
                for qc in range(QC):
                    nc.tensor.matmul(pss[qc], r(ones_row[0:1, :]),
                                     r(bsout_sb[0:1, dh * T:(dh + 1) * T]),
                                     start=False, stop=True,
                                     skip_group_check=True)
                    xin = xstr.tile([P, T], f32, tag="xstr")
                    nc.sync.dma_start(
                        out=xin, in_=xkv[qc * P:(qc + 1) * P, dh * T:(dh + 1) * T])
                    nc.vector.scalar_tensor_tensor(
                        x1[qc][:, dh * T:(dh + 1) * T], pss[qc],
                        DSC / SSA, xin, ALU.mult, ALU.add)

            # ============ Phase D: cross-attention ============
            g2x, b2x = lng["ln2x"]
            xs2_p = [a512.tile([P, 2, T], fp8, tag="a512", name=f"xs2_p_{i}")
                     for i in range(DP)]
            for qc in range(QC):
                ln_tile(x1[qc], g2x, b2x,
                        [xs2_p[j // 2][:, j % 2, qc * P:(qc + 1) * P]
                         for j in range(DT)])

            # cq pairs (feature-major)
            wqc_st = wst.tile([P, DT, DP, 2, P], fp8, tag="wst", name="wqc_st")
            nc.sync.dma_start(out=wqc_st, in_=wq_pk[:, :].rearrange(
                "p (j i t c) -> p j i t c", j=DT, i=DP, t=2))
            cq_p = [a512.tile([P, 2, T], fp8, tag="a512", name=f"cq_p_{i}")
                    for i in range(DP)]
            for j in range(DT):
                ps0 = pmm.tile([P, T], f32, tag="pmm")
                for i in range(DP):
                    nc.tensor.matmul(ps0, wqc_st[:, j, i, :, :], xs2_p[i],
                                     start=(i == 0), stop=(i == DP - 1),
                                     perf_mode=DR)
                nc.scalar.activation(cq_p[j // 2][:, j % 2, :], ps0,
                                     AF.Identity, scale=DSC,
                                     bias=bq_sb[:, j:j + 1])

            # cross scores (transposed), masked exp, sum
            exc = expp.tile([P, 2, T], fp8, tag="expp", name="exc")
            ps_sum = pmm.tile([P, T], f32, tag="pmm")
            for c in range(CC):
                ps_a = pmm.tile([P, T], f32, tag="pmm")
                for i in range(DP):
                    nc.tensor.matmul(ps_a, ck_p[i][:, :, c * P:(c + 1) * P],
                                     cq_p[i],
                                     start=(i == 0), stop=(i == DP - 1),
                                     perf_mode=DR)
                nc.scalar.activation(exc[:, c, :], ps_a, AF.Exp,
                                     scale=1.0 / 32.0)
                nc.vector.tensor_scalar_mul(exc[:, c, :], exc[:, c, :],
                                            mask_sb[:, c:c + 1])
                nc.tensor.matmul(ps_sum[0:1, :], ones_col8, exc[:, c, :],
                                 start=(c == 0), stop=(c == CC - 1))
            recip = rcp.tile([1, T], f32r, tag="recip")
            with nc.allow_low_precision(reason="softmax 1/sum in f32r"):
                nc.vector.reciprocal(recip, ps_sum[0:1, :])
            ps_rb = pmm.tile([P, T], f32, tag="pmm")
            nc.tensor.matmul(ps_rb, r(ssa_row), r(recip), start=True, stop=True)
            rb_c = rcp.tile([P, T], f32, tag="rb")
            nc.vector.tensor_scalar(rb_c, ps_rb, 1.0, None, ALU.mult)

            # cross AV -> cross pairs (feature-major fp8, scaled SSA)
            cross_p = [a512.tile([P, 2, T], fp8, tag="a512", name=f"cross_p_{i}")
                       for i in range(DP)]
            for j in range(DT):
                ps_c = pmm.tile([P, T], f32, tag="pmm")
                nc.tensor.matmul(ps_c, cv_p[:, :, j * P:(j + 1) * P], exc,
                                 start=True, stop=True, perf_mode=DR)
                dst = cross_p[j // 2][:, j % 2, :]
                nc.vector.tensor_tensor(dst, ps_c, rb_c, ALU.mult)

            # wo proj, gate, residual -> x2 (in place over x1)
            wo_st = wst.tile([P, 2, DP, 2, 4 * P], fp8, tag="wst", name="wo_st")
            nc.sync.dma_start(out=wo_st, in_=wo_pk[:, :].rearrange(
                "p (h i t c) -> p h i t c", h=2, i=DP, t=2))
            x2 = x1
            for dh in range(2):
                pss = [pmm.tile([P, T], f32, tag="pmm", name=f"pss_{qc}")
                       for qc in range(QC)]
                for i in range(DP):
                    for qc in range(QC):
                        nc.tensor.matmul(pss[qc],
                                         cross_p[i][:, :, qc * P:(qc + 1) * P],
                                         wo_st[:, dh, i, :, :],
                                         start=(i == 0), stop=False,
                                         perf_mode=DR)
                for qc in range(QC):
                    nc.tensor.matmul(pss[qc], r(ones_row[0:1, :]),
                                     r(bo_sb[0:1, dh * T:(dh + 1) * T]),
                                     start=False, stop=True,
                                     skip_group_check=True)
                    sl = (slice(None), slice(dh * T, (dh + 1) * T))
                    nc.vector.scalar_tensor_tensor(
                        x2[qc][sl], pss[qc], t_col, x1[qc][sl],
                        ALU.mult, ALU.add)

            # ============ Phase E: FFN ============
            g3, b3 = lng["ln3"]
            xs3_p = [a512.tile([P, 2, T], fp8, tag="a512", name=f"xs3_p_{i}")
                     for i in range(DP)]
            for qc in range(QC):
                ln_tile(x2[qc], g3, b3,
                        [xs3_p[j // 2][:, j % 2, qc * P:(qc + 1) * P]
                         for j in range(DT)])

            # up-proj + gelu -> h pair tiles (fp8); W = hi + lo, one group
            h_p = [hp.tile([P, 2, T], fp8, tag="hp", name=f"h_p_{g}")
                   for g in range(FP)]
            FB = 4                       # f-tiles per strip
            for fb in range(FT // FB):   # 8 strips of 4 f-tiles
                w1_st = wst.tile([P, FB, 2, DP, 2, P], fp8, tag="wst",
                                 name="w1_st")
                blk = FB * DP * 2 * P    # 4096 bytes per (f-block, hi/lo)
                nc.sync.dma_start(
                    out=w1_st[:, :, 0, :, :, :],
                    in_=w1_pk[:, fb * blk:(fb + 1) * blk]
                    .rearrange("p (f i t c) -> p f i t c", f=FB, i=DP, t=2))
                nc.sync.dma_start(
                    out=w1_st[:, :, 1, :, :, :],
                    in_=w1_pk[:, W1_SZ + fb * blk:W1_SZ + (fb + 1) * blk]
                    .rearrange("p (f i t c) -> p f i t c", f=FB, i=DP, t=2))
                for fi in range(FB):
                    f = fb * FB + fi
                    ps0 = pmm.tile([P, T], f32, tag="pmm")
                    for hl in range(2):
                        for i in range(DP):
                            nc.tensor.matmul(ps0, w1_st[:, fi, hl, i, :, :],
                                             xs3_p[i],
                                             start=(hl == 0 and i == 0),
                                             stop=(hl == 1 and i == DP - 1),
                                             perf_mode=DR)
                    nc.scalar.activation(h_p[f // 2][:, f % 2, :], ps0,
                                         gelu_func, scale=DSC,
                                         bias=b1_sb[:, f:f + 1])

            # down-proj + residual -> out
            GB = 4                       # g-pairs per strip
            for dh in range(2):
                pss = [pmm.tile([P, T], f32, tag="pmm", name=f"pss_{qc}")
                       for qc in range(QC)]
                for gb in range(FP // GB):   # 4 strips of 4 g-pairs
                    w2_st = wst.tile([P, GB, 2, 2, 4 * P], fp8, tag="wst",
                                     name="w2_st")
                    blk = GB * 2 * 4 * P     # 4096 bytes per (g-block, hi/lo)
                    base = dh * FP * 2 * 4 * P
                    nc.sync.dma_start(
                        out=w2_st[:, :, 0, :, :],
                        in_=w2_pk[:, base + gb * blk:base + (gb + 1) * blk]
                        .rearrange("p (g t c) -> p g t c", g=GB, t=2))
                    nc.sync.dma_start(
                        out=w2_st[:, :, 1, :, :],
                        in_=w2_pk[:, W2_SZ + base + gb * blk:W2_SZ + base + (gb + 1) * blk]
                        .rearrange("p (g t c) -> p g t c", g=GB, t=2))
                    for gi in range(GB):
                        g = gb * GB + gi
                        for hl in range(2):
                            for qc in range(QC):
                                nc.tensor.matmul(
                                    pss[qc],
                                    h_p[g][:, :, qc * P:(qc + 1) * P],
                                    w2_st[:, gi, hl, :, :],
                                    start=(gb == 0 and gi == 0 and hl == 0),
                                    stop=False,
                                    perf_mode=DR)
                for qc in range(QC):
                    nc.tensor.matmul(pss[qc], r(ones_row[0:1, :]),
                                     r(b2_sb[0:1, dh * T:(dh + 1) * T]),
                                     start=False, stop=True,
                                     skip_group_check=True)
                    ot = osb.tile([P, T], f32, tag="osb")
                    nc.vector.scalar_tensor_tensor(
                        ot, pss[qc], DSC, x2[qc][:, dh * T:(dh + 1) * T],
                        ALU.mult, ALU.add)
                    nc.sync.dma_start(
                        out=out[qc * P:(qc + 1) * P, dh * T:(dh + 1) * T], in_=ot)

    if compile_hw:
        nc.compile()
    return nc


def _fp8(a):
    return np.ascontiguousarray(a).astype(ml_dtypes.float8_e4m3)


def _pack_lhsT(wT, sw=SW):
    """W^T [Din, Dout] f32 -> fp8 pack [128, Dout/128 * Din/256 * 2 * 128]
    laid out as [p, j, i, plane, col] (j = out tile, i = contraction pair)."""
    din, dout = wT.shape
    a = (wT * sw).reshape(din // 256, 2, 128, dout // 128, 128)
    a = a.transpose(2, 3, 0, 1, 4)          # [p, j, i, plane, col]
    return _fp8(a.reshape(128, -1))


def _unpack_lhsT(pk, din, dout):
    a = pk.astype(np.float32).reshape(128, dout // 128, din // 256, 2, 128)
    return a.transpose(2, 3, 0, 1, 4).reshape(din, dout) / SW


def _pack_rhs(wT, sw=SW):
    """W^T [Din, Dout] f32 -> fp8 pack [128, Dout/512 * Din/256 * 2 * 512]
    laid out as [p, h, i, plane, col] (h = 512-col out block)."""
    din, dout = wT.shape
    a = (wT * sw).reshape(din // 256, 2, 128, dout // 512, 512)
    a = a.transpose(2, 3, 0, 1, 4)          # [p, h, i, plane, col]
    return _fp8(a.reshape(128, -1))


def _unpack_rhs(pk, din, dout):
    a = pk.astype(np.float32).reshape(128, dout // 512, din // 256, 2, 512)
    return a.transpose(2, 3, 0, 1, 4).reshape(din, dout) / SW


def _pack_lhsT_pair(wT):
    hi = _pack_lhsT(wT)
    lo = _pack_lhsT(wT - _unpack_lhsT(hi, *wT.shape))
    return np.concatenate([hi, lo], axis=1)


def _pack_rhs_pair(wT):
    hi = _pack_rhs(wT)
    lo = _pack_rhs(wT - _unpack_rhs(hi, *wT.shape))
    return np.concatenate([hi, lo], axis=1)


def make_in_maps(inputs):
    """Host-side sharding/layout prep. inputs: full arrays as in reference."""
    f = np.float32
    x = np.asarray(inputs["x"], f)
    cond = np.asarray(inputs["cond"], f)
    cmask = np.asarray(inputs["cond_mask"])
    g = lambda k: np.ascontiguousarray(np.asarray(inputs[k], f))
    tr = lambda k: np.ascontiguousarray(np.asarray(inputs[k], f).T)

    w_qkvT = tr("sa_in_w")                   # [D, 3D]
    wqkv_pk = np.concatenate(
        [_pack_lhsT(w_qkvT[:, D:2 * D]),      # K
         _pack_lhsT(w_qkvT[:, 0:D]),          # Q
         _pack_rhs(w_qkvT[:, 2 * D:3 * D])],  # V
        axis=1)

    # fold the v biases into the following projection biases (exact: the
    # softmax weights sum to 1), pre-scale rank-1 bias rows by SW*SSA
    bv_sa = g("sa_in_b")[2 * D:3 * D]
    bsout_eff = (np.asarray(inputs["sa_out_w"], f) @ bv_sa
                 + g("sa_out_b")) * (SW * SSA)
    bo_eff = (np.asarray(inputs["wo"], f) @ g("bv")
              + g("bo")) * (SW * SSA)

    shared = {
        "wqkv_pk": wqkv_pk,
        "b_qkv": g("sa_in_b").reshape(3 * D, 1),
        "wsout_pk": _pack_rhs(tr("sa_out_w")),
        "b_sout": bsout_eff.reshape(1, D),
        "wq_pk": _pack_lhsT(tr("wq")), "bq": g("bq").reshape(D, 1),
        "wk_pk": _pack_lhsT(tr("wk")), "bk": g("bk").reshape(D, 1),
        "wv_pk": _pack_rhs(tr("wv")),
        "wo_pk": _pack_rhs(tr("wo")), "bo": bo_eff.reshape(1, D),
        "gate": g("gate").reshape(1, 1),
        "w1_pk": _pack_lhsT_pair(tr("w1")),
        "b1": g("b1").reshape(FF, 1),
        "w2_pk": _pack_rhs_pair(tr("w2")),
        "b2": (g("b2") * SW).reshape(1, D),
        "ln1_g": g("ln1_g").reshape(1, D), "ln1_b": g("ln1_b").reshape(1, D),
        "ln2x_g": g("ln2x_g").reshape(1, D), "ln2x_b": g("ln2x_b").reshape(1, D),
        "ln2c_g": g("ln2c_g").reshape(1, D), "ln2c_b": g("ln2c_b").reshape(1, D),
        "ln3_g": g("ln3_g").reshape(1, D), "ln3_b": g("ln3_b").reshape(1, D),
    }
    in_maps = []
    for c in range(N_CORES):
        b, half = c // 2, c % 2
        loc = x[b, half * T:(half + 1) * T]
        oth = x[b, (1 - half) * T:(2 - half) * T]
        m = dict(shared)
        m["xkv"] = np.ascontiguousarray(np.concatenate([loc, oth], axis=0))
        m["cond"] = np.ascontiguousarray(cond[b])
        m["maskmul"] = (cmask[b] != 0).astype(f).reshape(SC, 1)
        in_maps.append(m)
    return in_maps


_CACHED_NC = None


def kernel(**inputs):
    from concourse.bass_utils import run_bass_kernel_spmd
    global _CACHED_NC
    if _CACHED_NC is None:
        _CACHED_NC = build_nc(compile_hw=True)
    in_maps = make_in_maps(inputs)
    res = run_bass_kernel_spmd(_CACHED_NC, in_maps, list(range(N_CORES)))
    out = np.empty((B, S, D), np.float32)
    for c in range(N_CORES):
        b, half = c // 2, c % 2
        out[b, half * T:(half + 1) * T] = res.results[c]["out"]
    return out
